# revision 7
# baseline (speedup 1.0000x reference)
"""Trainium2 Bass kernel for nn_Attention_15109694948028.

Single-layer attention block: QKV proj -> 8-head SDPA (S=4096, d_k=64)
-> out proj -> residual -> LayerNorm.  fp32 I/O.

Strategy: sequence-parallel across 8 NeuronCores.  Core i computes the
full output for query rows [i*512, (i+1)*512) of all 8 heads; K and V
are computed redundantly on every core (cheap: 2 x 512^2 x 4096 MACs),
so there are NO collectives -- cores are fully independent and the host
just concatenates the 8 output slices.

On-core dataflow (all matmuls in float32r = full-rate TF32-class,
host pre-rounds inputs with the exact HW rounding so PE results are
fp32-exact on the rounded operands):

  phase A: QT per head-pair [128, 512] from a per-core pre-sliced
           xq = x^T[:, islice] input.
  half loop (k in two halves of 2048, bounds SBUF):
    stream x^T chunks [128,512]; K^T half per head-pair [128, 2048] and
    V' half per k-tile [128, 8*65] (ones column per head folds the
    softmax row-sum into the ctx matmul);
    per head: scores^T tiles ST[k,q] = KT_h^T @ QT_h on PE, exp on
    ScalarE (scale=1/8 folds 1/sqrt(d_k); no max-subtract: scores are
    ~N(0,1) so exp cannot overflow), ctx^T accumulated in PSUM over the
    half's 16 k-tiles, then merged into an SBUF accumulator.
  normalize: row 64 of ctx accumulator = softmax denominators; divide
           via DVE reciprocal + gpsimd partition_broadcast.
  phase D: out proj accumulates all 8 heads per q-subtile in PSUM,
           + residual + LayerNorm on DVE/ACT, DMA out.
"""

import numpy as np

import concourse.bacc as bacc
import concourse.tile as tile
from concourse import mybir
from concourse.bass_utils import run_bass_kernel_spmd

f32 = mybir.dt.float32
f32r = mybir.dt.float32r
AF = mybir.ActivationFunctionType

S = 4096
D = 512
H = 8
DK = 64
NCORES = 8
SLICE = S // NCORES          # 512 query rows per core
P = 128                      # partitions
HALF = S // 2                # 2048 k rows per half
NKTH = HALF // P             # 16 k-tiles per half
NCHH = HALF // 512           # 4 x^T chunks per half
NQS = SLICE // P             # 4 q-subtiles
KGRP = 2                     # k-tiles per exp batch (2 PSUM banks)
EPS = 1e-5


def _round_f32r(a: np.ndarray) -> np.ndarray:
    """RNE to 11 explicit mantissa bits (matches HW fp32 -> f32r cast)."""
    b = np.ascontiguousarray(a, dtype=np.float32).view(np.uint32)
    lsb = (b >> np.uint32(12)) & np.uint32(1)
    return ((b + np.uint32(0x7FF) + lsb) & np.uint32(0xFFFFF000)).view(np.float32)


def _build_nc(has_bias: bool, has_bo: bool, has_gamma: bool, has_beta: bool):
    nc = bacc.Bacc("TRN2", target_bir_lowering=False, debug=False)

    xT = nc.dram_tensor("xT", [D + 1, S], f32r, kind="ExternalInput")
    wq = nc.dram_tensor("wq", [D + 1, D], f32r, kind="ExternalInput")
    wk = nc.dram_tensor("wk", [D + 1, D], f32r, kind="ExternalInput")
    wv = nc.dram_tensor("wv", [D + 1, D], f32r, kind="ExternalInput")
    wo = nc.dram_tensor("wo", [D, D], f32r, kind="ExternalInput")
    xq = nc.dram_tensor("xq", [D + 1, SLICE], f32r, kind="ExternalInput")
    xs = nc.dram_tensor("x_slice", [SLICE, D], f32, kind="ExternalInput")
    bo = nc.dram_tensor("bo", [1, D], f32, kind="ExternalInput")
    gamma = nc.dram_tensor("gamma", [1, D], f32, kind="ExternalInput")
    beta = nc.dram_tensor("beta", [1, D], f32, kind="ExternalInput")
    y = nc.dram_tensor("y", [SLICE, D], f32, kind="ExternalOutput")

    with tile.TileContext(nc) as tc:
        with (
            tc.tile_pool(name="qt", bufs=1) as qtp,
            tc.tile_pool(name="cacc", bufs=1) as cap,
            tc.tile_pool(name="consts", bufs=1) as cp,
        ):
            eps_t = cp.tile([P, 1], f32, tag="eps")
            nc.gpsimd.memset(eps_t[:], EPS)

            def bcast_row(dram_row, tag):
                r = cp.tile([1, D], f32, tag=f"{tag}_row", name=f"{tag}_row")
                nc.sync.dma_start(r[:], dram_row)
                b = cp.tile([P, D], f32, tag=f"{tag}_b", name=f"{tag}_b")
                nc.gpsimd.partition_broadcast(b[:], r[0:1, :])
                return b

            bo_b = bcast_row(bo[:], "bo") if has_bo else None
            gamma_b = bcast_row(gamma[:], "gamma") if has_gamma else None
            beta_b = bcast_row(beta[:], "beta") if has_beta else None
            if has_bias:
                xt_ones = cp.tile([1, S], f32r, tag="xt_ones")
                nc.sync.dma_start(xt_ones[:], xT[D:D + 1, :])

            qtS = [qtp.tile([P, SLICE], f32r, tag=f"qt{p}", name=f"qt{p}")
                   for p in range(4)]
            caccS = [cap.tile([65, SLICE], f32, tag=f"cacc{h}", name=f"cacc{h}")
                     for h in range(H)]

            # ---- phase A: Q projection from per-core xq ----
            with (
                tc.tile_pool(name="wqp", bufs=1) as wqp,
                tc.tile_pool(name="xqp", bufs=1) as xqp,
                tc.tile_pool(name="psA", bufs=4, space="PSUM") as psA,
            ):
                wqS = [wqp.tile([P, D], f32r, tag=f"wq{e}", name=f"wq{e}")
                       for e in range(4)]
                for e in range(4):
                    nc.sync.dma_start(wqS[e][:], wq[e * P:(e + 1) * P, :])
                xqS = [xqp.tile([P, SLICE], f32r, tag=f"xq{e}", name=f"xq{e}")
                       for e in range(4)]
                for e in range(4):
                    nc.sync.dma_start(xqS[e][:], xq[e * P:(e + 1) * P, :])
                if has_bias:
                    wqb = wqp.tile([1, D], f32r, tag="wqb")
                    nc.sync.dma_start(wqb[:], wq[D:D + 1, :])
                    xq_ones = wqp.tile([1, SLICE], f32r, tag="xq_ones")
                    nc.sync.dma_start(xq_ones[:], xq[D:D + 1, :])
                for p in range(4):
                    ps = psA.tile([P, SLICE], f32, tag="psA")
                    for e in range(4):
                        nc.tensor.matmul(
                            ps[:], wqS[e][:, p * P:(p + 1) * P], xqS[e][:],
                            start=(e == 0), stop=(e == 3 and not has_bias),
                        )
                    if has_bias:
                        nc.tensor.matmul(
                            ps[:], wqb[0:1, p * P:(p + 1) * P], xq_ones[:],
                            start=False, stop=True,
                        )
                    nc.vector.tensor_copy(qtS[p][:], ps[:])

            # ---- half loop: K/V proj for half + attention for half ----
            with (
                tc.tile_pool(name="wkv", bufs=1) as wkvp,
                tc.tile_pool(name="kth", bufs=1) as kthp,
                tc.tile_pool(name="vph", bufs=1) as vphp,
                tc.tile_pool(name="xtc", bufs=2) as xtcp,
                tc.tile_pool(name="es", bufs=3) as esp,
                tc.tile_pool(name="psB", bufs=2, space="PSUM") as psB,
                tc.tile_pool(name="st", bufs=2, space="PSUM") as stp,
                tc.tile_pool(name="ctxps", bufs=2, space="PSUM") as cpp,
            ):
                wkS = [wkvp.tile([P, D], f32r, tag=f"wk{e}", name=f"wk{e}")
                       for e in range(4)]
                wvS = [wkvp.tile([P, D], f32r, tag=f"wv{e}", name=f"wv{e}")
                      for e in range(4)]
                for e in range(4):
                    nc.sync.dma_start(wkS[e][:], wk[e * P:(e + 1) * P, :])
                    nc.sync.dma_start(wvS[e][:], wv[e * P:(e + 1) * P, :])
                if has_bias:
                    wkb = wkvp.tile([1, D], f32r, tag="wkb")
                    wvb = wkvp.tile([1, D], f32r, tag="wvb")
                    nc.sync.dma_start(wkb[:], wk[D:D + 1, :])
                    nc.sync.dma_start(wvb[:], wv[D:D + 1, :])

                ktS = [kthp.tile([P, HALF], f32r, tag=f"kt{p}", name=f"kt{p}")
                       for p in range(4)]
                vpS = [vphp.tile([P, H * 65], f32r, tag=f"vp{t}", name=f"vp{t}")
                       for t in range(NKTH)]

                for half in range(2):
                    k0 = half * HALF
                    # K/V projection for this half, streaming xT chunks
                    for cc in range(NCHH):
                        c0 = k0 + cc * 512
                        xc = [xtcp.tile([P, 512], f32r, tag=f"xtc{e}",
                                        name=f"xtc{e}") for e in range(4)]
                        for e in range(4):
                            nc.sync.dma_start(
                                xc[e][:], xT[e * P:(e + 1) * P, c0:c0 + 512])
                        for p in range(4):
                            ps = psB.tile([P, 512], f32, tag="psB")
                            for e in range(4):
                                nc.tensor.matmul(
                                    ps[:], wkS[e][:, p * P:(p + 1) * P], xc[e][:],
                                    start=(e == 0),
                                    stop=(e == 3 and not has_bias),
                                )
                            if has_bias:
                                nc.tensor.matmul(
                                    ps[:], wkb[0:1, p * P:(p + 1) * P],
                                    xt_ones[0:1, c0:c0 + 512],
                                    start=False, stop=True,
                                )
                            nc.vector.tensor_copy(
                                ktS[p][:, cc * 512:(cc + 1) * 512], ps[:])
                        for j in range(4):
                            lt = cc * 4 + j  # local s-tile in half
                            ps = psB.tile([P, D], f32, tag="psB")
                            for e in range(4):
                                nc.tensor.matmul(
                                    ps[:], xc[e][:, j * P:(j + 1) * P], wvS[e][:],
                                    start=(e == 0),
                                    stop=(e == 3 and not has_bias),
                                )
                            if has_bias:
                                nc.tensor.matmul(
                                    ps[:],
                                    xt_ones[0:1, c0 + j * P:c0 + (j + 1) * P],
                                    wvb[:], start=False, stop=True,
                                )
                            vt = vpS[lt]
                            v3 = vt.rearrange("p (h c) -> p h c", c=65)
                            nc.gpsimd.memset(v3[:, :, 64:65].bitcast(f32), 1.0)
                            nc.vector.tensor_copy(
                                v3[:, :, 0:64],
                                ps[:].rearrange("p (h d) -> p h d", d=DK))

                    # attention for this half
                    for h in range(H):
                        p, off = h // 2, (h % 2) * DK
                        ctxu = cpp.tile([65, SLICE], f32, tag="ctxu")
                        for g in range(NKTH // KGRP):
                            stt = stp.tile([P, KGRP * 512], f32, tag="st")
                            for j in range(KGRP):
                                lkt = g * KGRP + j
                                nc.tensor.matmul(
                                    stt[:, j * 512:(j + 1) * 512],
                                    ktS[p][off:off + DK, lkt * P:(lkt + 1) * P],
                                    qtS[p][off:off + DK, :],
                                    start=True, stop=True,
                                )
                            es = esp.tile([P, KGRP * 512], f32r, tag="es")
                            nc.scalar.activation(es[:], stt[:], AF.Exp,
                                                 scale=0.125)
                            for j in range(KGRP):
                                lkt = g * KGRP + j
                                nc.tensor.matmul(
                                    ctxu[:],
                                    vpS[lkt][:, h * 65:(h + 1) * 65],
                                    es[:, j * 512:(j + 1) * 512],
                                    start=(lkt == 0), stop=(lkt == NKTH - 1),
                                )
                        if half == 0:
                            nc.vector.tensor_copy(caccS[h][:], ctxu[:])
                        else:
                            nc.vector.tensor_add(caccS[h][:], ctxu[:],
                                                 caccS[h][:])

            # ---- normalize: ctx[d, q] / rowsum[q] -> ctxT (f32r) ----
            with tc.tile_pool(name="nrm", bufs=1) as nrmp, \
                 tc.tile_pool(name="ctxT", bufs=1) as ctp:
                ctxT = [ctp.tile([DK, SLICE], f32r, tag=f"ctxT{h}",
                                 name=f"ctxT{h}") for h in range(H)]
                for h in range(H):
                    rs0 = nrmp.tile([1, SLICE], f32, tag="rs0", bufs=2)
                    nc.sync.dma_start(rs0[:], caccS[h][64:65, :])
                    rc = nrmp.tile([1, SLICE], f32, tag="rc", bufs=2)
                    scr = nrmp.tile([1, SLICE], f32, tag="scr", bufs=2)
                    nc.vector.reciprocal_approx_accurate(rc[:], rs0[:], scr[:])
                    bc = nrmp.tile([DK, SLICE], f32, tag="bc", bufs=2)
                    nc.gpsimd.partition_broadcast(bc[:], rc[0:1, :])
                    nc.vector.tensor_mul(ctxT[h][:], caccS[h][0:DK, :], bc[:])

                # ---- phase D: out proj + residual + LayerNorm ----
                with (
                    tc.tile_pool(name="wop", bufs=1) as wop,
                    tc.tile_pool(name="psD", bufs=4, space="PSUM") as psD,
                    tc.tile_pool(name="ln", bufs=2) as lnp,
                ):
                    woS = [wop.tile([DK, D], f32r, tag=f"wo{h}", name=f"wo{h}")
                           for h in range(H)]
                    for h in range(H):
                        nc.sync.dma_start(woS[h][:], wo[h * DK:(h + 1) * DK, :])
                    for qs in range(NQS):
                        op = psD.tile([P, D], f32, tag="psD")
                        for h in range(H):
                            nc.tensor.matmul(
                                op[:], ctxT[h][:, qs * P:(qs + 1) * P],
                                woS[h][:],
                                start=(h == 0), stop=(h == H - 1),
                            )
                        xt_ = lnp.tile([P, D], f32, tag="xres")
                        nc.sync.dma_start(xt_[:], xs[qs * P:(qs + 1) * P, :])
                        t = lnp.tile([P, D], f32, tag="t")
                        nc.vector.tensor_add(t[:], op[:], xt_[:])
                        if has_bo:
                            nc.vector.tensor_add(t[:], t[:], bo_b[:])
                        s1 = lnp.tile([P, 1], f32, tag="s1")
                        nc.vector.reduce_sum(s1[:], t[:],
                                             axis=mybir.AxisListType.X)
                        negmu = lnp.tile([P, 1], f32, tag="negmu")
                        nc.vector.tensor_scalar_mul(negmu[:], s1[:], -1.0 / D)
                        tcen = lnp.tile([P, D], f32, tag="tcen")
                        nc.vector.tensor_scalar_add(tcen[:], t[:], negmu[:])
                        sq = lnp.tile([P, D], f32, tag="sq")
                        v1 = lnp.tile([P, 1], f32, tag="v1")
                        nc.scalar.activation(sq[:], tcen[:], AF.Square,
                                             accum_out=v1[:])
                        std = lnp.tile([P, 1], f32, tag="std")
                        nc.scalar.activation(std[:], v1[:], AF.Sqrt,
                                             bias=eps_t[:], scale=1.0 / D)
                        rstd = lnp.tile([P, 1], f32, tag="rstd")
                        nc.vector.reciprocal(rstd[:], std[:])
                        out_t = lnp.tile([P, D], f32, tag="out_t")
                        nc.vector.tensor_scalar_mul(out_t[:], tcen[:], rstd[:])
                        if has_gamma:
                            nc.vector.tensor_mul(out_t[:], out_t[:], gamma_b[:])
                        if has_beta:
                            nc.vector.tensor_add(out_t[:], out_t[:], beta_b[:])
                        nc.sync.dma_start(y[qs * P:(qs + 1) * P, :], out_t[:])

    nc.compile()
    return nc


_NC_CACHE: dict = {}


def _get_nc(flags):
    if flags not in _NC_CACHE:
        _NC_CACHE[flags] = _build_nc(*flags)
    return _NC_CACHE[flags]


def _prep_inputs(inputs):
    """Build the 8 per-core input maps from the full problem inputs."""
    x = np.ascontiguousarray(np.asarray(inputs["x"], dtype=np.float32))
    Wq = np.asarray(inputs["Wq"], dtype=np.float32)
    Wk = np.asarray(inputs["Wk"], dtype=np.float32)
    Wv = np.asarray(inputs["Wv"], dtype=np.float32)
    Wo = np.asarray(inputs["Wo"], dtype=np.float32)
    bq = np.asarray(inputs["bq"], dtype=np.float32)
    bk = np.asarray(inputs["bk"], dtype=np.float32)
    bv = np.asarray(inputs["bv"], dtype=np.float32)
    bo = np.asarray(inputs["bo"], dtype=np.float32)
    gamma = np.asarray(inputs["gamma"], dtype=np.float32)
    beta = np.asarray(inputs["beta"], dtype=np.float32)

    has_bias = bool(np.any(bq) or np.any(bk) or np.any(bv))
    has_bo = bool(np.any(bo))
    has_gamma = bool(np.any(gamma != 1.0))
    has_beta = bool(np.any(beta))
    flags = (has_bias, has_bo, has_gamma, has_beta)

    xT = np.concatenate([x.T, np.ones((1, S), np.float32)], axis=0)
    xT = _round_f32r(xT)
    wq_e = _round_f32r(np.concatenate([Wq, bq[None, :]], axis=0))
    wk_e = _round_f32r(np.concatenate([Wk, bk[None, :]], axis=0))
    wv_e = _round_f32r(np.concatenate([Wv, bv[None, :]], axis=0))
    wo_r = _round_f32r(Wo)

    shared = {
        "xT": xT, "wq": wq_e, "wk": wk_e, "wv": wv_e, "wo": wo_r,
        "bo": bo.reshape(1, D), "gamma": gamma.reshape(1, D),
        "beta": beta.reshape(1, D),
    }
    in_maps = []
    for i in range(NCORES):
        m = dict(shared)
        m["xq"] = np.ascontiguousarray(xT[:, i * SLICE:(i + 1) * SLICE])
        m["x_slice"] = np.ascontiguousarray(x[i * SLICE:(i + 1) * SLICE, :])
        in_maps.append(m)
    return flags, in_maps


def _run(inputs, trace=False, **kw):
    flags, in_maps = _prep_inputs(inputs)
    nc = _get_nc(flags)
    res = run_bass_kernel_spmd(nc, in_maps, core_ids=list(range(NCORES)),
                               trace=trace, **kw)
    out = np.concatenate([res.results[i]["y"] for i in range(NCORES)], axis=0)
    return out, res


def kernel(**inputs) -> np.ndarray:
    out, _ = _run(inputs, trace=False)
    return out
